# revision 13
# baseline (speedup 1.0000x reference)
"""GRU decoder kernel for trn2, 8-core data-parallel over batch.

Reference semantics (per step, torch-GRU style):
    gi = x @ W_ih.T + b_ih ; gh = h @ W_hh.T + b_hh
    r = sigmoid(gi_r + gh_r); z = sigmoid(gi_z + gh_z)
    n = tanh(gi_n + r * gh_n)
    h' = (1-z)*n + z*h ; out = h' @ W_out.T + b_out  (out feeds back as next x)

Design notes:
 - batch sharded 8 ways (32 rows/core), weights replicated per core in SBUF.
 - batch-major matmuls: lhsT = x.T/h.T chunks (stationary), rhs = weight
   matrices streamed (moving).  float32r dtype -> 1 cycle/row on PE.
 - r,z gates accumulate gi+gh directly in PSUM; n-gate parts kept separate.
 - h' and out are transposed back to feature-major via PE transpose for the
   next step's stationary operands.
"""

import sys

sys.path.insert(0, "/opt/trn_rl_repo")
sys.path.insert(0, "/opt/trn_rl_repo/concourse")

import numpy as np

N_CORES = 8
P = 128  # partitions


def _build(T, B_loc, L, H, O, has_bias):
    import concourse.bacc as bacc
    import concourse.bass as bass
    import concourse.mybir as mybir
    import concourse.tile as tile
    from concourse.masks import make_identity

    f32 = mybir.dt.float32
    f32r = mybir.dt.float32r
    H3 = 3 * H
    KH = H // P  # 8 h chunks
    KO = O // P  # 2 x chunks
    UNROLL = 8
    assert T % UNROLL == 0

    nc = bacc.Bacc("TRN2", target_bir_lowering=False, debug=False,
                   num_devices=N_CORES)

    # ---- DRAM I/O ----
    zT_d = nc.dram_tensor("zT", [L, B_loc], f32r, kind="ExternalInput")
    WlhT_d = nc.dram_tensor("WlhT", [P, L // P, H], f32r, kind="ExternalInput")
    WihT_d = nc.dram_tensor("WihT", [P, KO, H3], f32r, kind="ExternalInput")
    WhhT_d = nc.dram_tensor("WhhT", [P, KH, H3], f32r, kind="ExternalInput")
    WoutT_d = nc.dram_tensor("WoutT", [P, KH, O], f32r, kind="ExternalInput")
    if has_bias:
        # [b_rz (2H) | b_inn (H) | b_hn (H) | b_out (O) | b_lh (H)] combined row
        bias_d = nc.dram_tensor("bias", [1, 2 * H + H + H + O + H], f32r,
                                kind="ExternalInput")
    y_d = nc.dram_tensor("y", [B_loc, T, O], f32, kind="ExternalOutput")

    with tile.TileContext(nc) as tc:
        # ---- persistent SBUF ----
        WlhT = nc.alloc_sbuf_tensor("WlhT_s", [P, L // P, H], f32r)
        WihT = nc.alloc_sbuf_tensor("WihT_s", [P, KO, H3], f32r)
        WhhT = nc.alloc_sbuf_tensor("WhhT_s", [P, KH, H3], f32r)
        WoutT = nc.alloc_sbuf_tensor("WoutT_s", [P, KH, O], f32r)
        zT = nc.alloc_sbuf_tensor("zT_s", [L, B_loc], f32r)
        hT = nc.alloc_sbuf_tensor("hT_s", [P, KH, B_loc], f32r)  # h.T chunks
        xT = nc.alloc_sbuf_tensor("xT_s", [P, KO, B_loc], f32r)  # x.T chunks
        h_bm = nc.alloc_sbuf_tensor("h_bm_s", [B_loc, H], f32)   # batch-major h
        ybuf = nc.alloc_sbuf_tensor("ybuf_s", [B_loc, UNROLL, O], f32)
        ident = nc.alloc_sbuf_tensor("ident_s", [B_loc, B_loc], f32)
        if has_bias:
            bias_s = nc.alloc_sbuf_tensor("bias_s", [1, 5 * H + O], f32r)
            ones_s = nc.alloc_sbuf_tensor("ones_s", [1, B_loc], f32r)

        for dram, sbuf in [(WlhT_d, WlhT), (WihT_d, WihT), (WhhT_d, WhhT),
                           (WoutT_d, WoutT), (zT_d, zT)]:
            nc.sync.dma_start(sbuf[:], dram[:])
        if has_bias:
            nc.sync.dma_start(bias_s[:], bias_d[:])
            nc.gpsimd.memset(ones_s[:], 1.0)
        make_identity(nc, ident[:])
        zero_sb = nc.alloc_sbuf_tensor("zero_s", [P, KO * B_loc], f32)
        nc.gpsimd.memset(zero_sb[:], 0.0)
        nc.vector.tensor_copy(xT[:], zero_sb[:])

        sb_pool = tc.alloc_tile_pool(name="sb", bufs=2)
        ps_pool = tc.alloc_tile_pool(name="ps", bufs=1, space="PSUM")
        acc_pool = tc.alloc_tile_pool(name="acc", bufs=4, space="PSUM")

        def bias_row(ps_chunk, col0, ncols, start):
            # ps_chunk += ones.T @ bias_row  (adds bias to every batch row)
            nc.tensor.matmul(ps_chunk,
                             ones_s[:, :],
                             bias_s[:, col0:col0 + ncols],
                             start=start, stop=False)

        # ---- h0 = tanh(z @ W_lh.T) ----
        for j in range(H // 512):
            cs = slice(j * 512, (j + 1) * 512)
            h0_ps = acc_pool.tile([B_loc, 512], f32, tag="acc")
            first = True
            if has_bias:
                bias_row(h0_ps[:], 4 * H + O + j * 512, 512, True)
                first = False
            nc.tensor.matmul(h0_ps[:], zT[:, :],
                             WlhT[:, 0, cs],
                             start=first, stop=True)
            nc.scalar.activation(h_bm[:, cs], h0_ps[:],
                                 mybir.ActivationFunctionType.Tanh)
        tp0 = ps_pool.tile([P, KH * B_loc], f32, tag="tp")
        for c in range(KH):
            nc.tensor.transpose(tp0[:, c * B_loc:(c + 1) * B_loc],
                                h_bm[:, c * P:(c + 1) * P], ident[:])
        nc.vector.tensor_copy(hT[:], tp0[:])

        # ---- the scan ----
        NC512 = H // 512  # 512-col accumulation chunks per gate

        def gate_mm(ps, wcol0, first):
            """accumulate x @ W_ih.T[:, wcol] + h @ W_hh.T[:, wcol] into ps"""
            wcs = slice(wcol0, wcol0 + 512)
            if has_bias:
                bias_row(ps, wcol0, 512, True)
                first = False
            for k in range(KO):
                nc.tensor.matmul(ps, xT[:, k, :],
                                 WihT[:, k, wcs],
                                 start=first and k == 0, stop=False)
            for k in range(KH):
                nc.tensor.matmul(ps, hT[:, k, :],
                                 WhhT[:, k, wcs],
                                 start=False, stop=k == KH - 1)

        def step(j, iv0):
            r_sb = sb_pool.tile([B_loc, H], f32, tag="r")
            z_sb = sb_pool.tile([B_loc, H], f32, tag="z")
            # r and z gates: gi+gh accumulated together, drained eagerly
            for g, gsb in ((0, r_sb), (1, z_sb)):
                for c in range(NC512):
                    cs = slice(c * 512, (c + 1) * 512)
                    ps = acc_pool.tile([B_loc, 512], f32, tag="acc")
                    gate_mm(ps[:], g * H + c * 512, True)
                    nc.scalar.activation(gsb[:, cs], ps[:],
                                         mybir.ActivationFunctionType.Sigmoid)
            # n gate: i_n and h_n kept separate per 512-chunk
            for c in range(NC512):
                cs = slice(c * 512, (c + 1) * 512)
                wcs = slice(2 * H + c * 512, 2 * H + (c + 1) * 512)
                inn = acc_pool.tile([B_loc, 512], f32, tag="acc")
                first = True
                if has_bias:
                    bias_row(inn[:], 2 * H + c * 512, 512, True)
                    first = False
                for k in range(KO):
                    nc.tensor.matmul(inn[:], xT[:, k, :],
                                     WihT[:, k, wcs],
                                     start=first and k == 0, stop=k == KO - 1)
                hn = acc_pool.tile([B_loc, 512], f32, tag="acc")
                first = True
                if has_bias:
                    bias_row(hn[:], 3 * H + c * 512, 512, True)
                    first = False
                for k in range(KH):
                    nc.tensor.matmul(hn[:], hT[:, k, :],
                                     WhhT[:, k, wcs],
                                     start=first and k == 0, stop=k == KH - 1)
                # n = tanh(inn + r*hn); h' = n + z*(h - n)
                t0 = sb_pool.tile([B_loc, 512], f32, tag="t0")
                nc.vector.tensor_mul(t0[:], r_sb[:, cs], hn[:])
                npre = sb_pool.tile([B_loc, 512], f32, tag="npre")
                nc.vector.tensor_add(npre[:], t0[:], inn[:])
                n_sb = sb_pool.tile([B_loc, 512], f32, tag="n")
                nc.scalar.activation(n_sb[:], npre[:],
                                     mybir.ActivationFunctionType.Tanh)
                d_sb = sb_pool.tile([B_loc, 512], f32, tag="d")
                nc.vector.tensor_sub(d_sb[:], h_bm[:, cs], n_sb[:])
                e_sb = sb_pool.tile([B_loc, 512], f32, tag="e")
                nc.vector.tensor_mul(e_sb[:], z_sb[:, cs], d_sb[:])
                nc.vector.tensor_add(h_bm[:, cs], n_sb[:], e_sb[:])

            # h'.T for next step
            tp = ps_pool.tile([P, KH * B_loc], f32, tag="tp")
            for c in range(KH):
                nc.tensor.transpose(tp[:, c * B_loc:(c + 1) * B_loc],
                                    h_bm[:, c * P:(c + 1) * P], ident[:])
            nc.vector.tensor_copy(hT[:], tp[:])

            # out = h' @ W_out.T  (also next x)
            out_ps = ps_pool.tile([B_loc, O], f32, tag="out")
            first = True
            if has_bias:
                bias_row(out_ps[:, :], 4 * H, O, True)
                first = False
            for k in range(KH):
                nc.tensor.matmul(out_ps[:], hT[:, k, :],
                                 WoutT[:, k, :],
                                 start=first and k == 0, stop=k == KH - 1)
            nc.vector.tensor_copy(ybuf[:, j, :], out_ps[:])
            # x.T for next step
            xp = ps_pool.tile([P, KO * B_loc], f32, tag="xp")
            for c in range(KO):
                nc.tensor.transpose(xp[:, c * B_loc:(c + 1) * B_loc],
                                    ybuf[:, j, c * P:(c + 1) * P], ident[:])
            nc.vector.tensor_copy(xT[:], xp[:])

        def block(iv0, unroll):
            assert unroll == UNROLL
            for j in range(UNROLL):
                step(j, iv0)
            nc.sync.dma_start(y_d[:, bass.ds(iv0, UNROLL), :], ybuf[:])

        tc.For_i_unrolled_general(
            start=0, end=T, step=1,
            unrollable_body=block, max_unroll=UNROLL,
            hint_engines=(mybir.EngineType.PE,),
        )
        acc_pool.release()
        ps_pool.release()
        sb_pool.release()

    nc.compile()
    return nc


_CACHE = {}


def _round_f32r(a):
    """Round-to-nearest-even to fp32r (11 mantissa bits kept)."""
    b = np.ascontiguousarray(a, dtype=np.float32).view(np.uint32)
    lsb = (b >> np.uint32(12)) & np.uint32(1)
    out = (b + np.uint32(0x7FF) + lsb) & np.uint32(0xFFFFF000)
    return out.view(np.float32)


def _prepare(z, W_lh, b_lh, W_ih, b_ih, W_hh, b_hh, W_out, b_out, seq_len):
    z = np.asarray(z, dtype=np.float32)
    W_lh = np.asarray(W_lh, dtype=np.float32)
    W_ih = np.asarray(W_ih, dtype=np.float32)
    W_hh = np.asarray(W_hh, dtype=np.float32)
    W_out = np.asarray(W_out, dtype=np.float32)
    b_lh = np.asarray(b_lh, dtype=np.float32)
    b_ih = np.asarray(b_ih, dtype=np.float32)
    b_hh = np.asarray(b_hh, dtype=np.float32)
    b_out = np.asarray(b_out, dtype=np.float32)
    T = int(seq_len)
    B, L = z.shape
    H = W_hh.shape[1]
    O = W_out.shape[0]
    H3 = 3 * H
    B_loc = B // N_CORES
    has_bias = any(np.any(b) for b in (b_lh, b_ih, b_hh, b_out))

    key = (T, B_loc, L, H, O, has_bias)
    if key not in _CACHE:
        _CACHE[key] = _build(*key)
    nc = _CACHE[key]

    def chunked(WT, k):  # [KD*P, N] -> [P, KD, N]
        KD = WT.shape[0] // P
        return np.ascontiguousarray(
            WT.reshape(KD, P, WT.shape[1]).transpose(1, 0, 2))

    common = {
        "WlhT": _round_f32r(chunked(W_lh.T, L // P)),
        "WihT": _round_f32r(chunked(W_ih.T, 2)),
        "WhhT": _round_f32r(chunked(W_hh.T, H // P)),
        "WoutT": _round_f32r(chunked(W_out.T, H // P)),
    }
    if has_bias:
        brz = (b_ih + b_hh)[:2 * H]
        common["bias"] = np.concatenate(
            [brz, b_ih[2 * H:], b_hh[2 * H:], b_out, b_lh]
        ).reshape(1, -1).astype(np.float32)

    in_maps = []
    for c in range(N_CORES):
        m = dict(common)
        m["zT"] = _round_f32r(z[c * B_loc:(c + 1) * B_loc].T)
        in_maps.append(m)
    return nc, in_maps


def kernel(**inputs):
    from concourse import bass_utils

    nc, in_maps = _prepare(**inputs)
    res = bass_utils.run_bass_kernel_spmd(
        nc, in_maps, core_ids=list(range(N_CORES)))
    y = np.concatenate([r["y"] for r in res.results], axis=0)
    return y


def run_traced(inputs):
    from concourse import bass_utils

    nc, in_maps = _prepare(**inputs)
    return bass_utils.run_bass_kernel_spmd(
        nc, in_maps, core_ids=list(range(N_CORES)), trace=True)


# revision 19
# speedup vs baseline: 1.0200x; 1.0200x over previous
"""GRU decoder kernel for trn2, 8-core data-parallel over batch.

Reference semantics (per step, torch-GRU style):
    gi = x @ W_ih.T + b_ih ; gh = h @ W_hh.T + b_hh
    r = sigmoid(gi_r + gh_r); z = sigmoid(gi_z + gh_z)
    n = tanh(gi_n + r * gh_n)
    h' = (1-z)*n + z*h ; out = h' @ W_out.T + b_out  (out feeds back as next x)

Design notes:
 - batch sharded 8 ways (32 rows/core), weights replicated per core in SBUF.
 - batch-major matmuls: lhsT = x.T/h.T chunks (stationary), rhs = weight
   matrices streamed (moving).  float32r dtype -> 1 cycle/row on PE.
 - r,z gates accumulate gi+gh directly in PSUM; n-gate parts kept separate.
 - h' and out are transposed back to feature-major via PE transpose for the
   next step's stationary operands.
"""

import sys

sys.path.insert(0, "/opt/trn_rl_repo")
sys.path.insert(0, "/opt/trn_rl_repo/concourse")

import numpy as np

N_CORES = 8
P = 128  # partitions


def _build(T, B_loc, L, H, O, has_bias, static=False, ablate=()):
    import concourse.bacc as bacc
    import concourse.bass as bass
    import concourse.mybir as mybir
    import concourse.tile as tile
    from concourse.masks import make_identity

    f32 = mybir.dt.float32
    f32r = mybir.dt.float32r
    H3 = 3 * H
    KH = H // P  # 8 h chunks
    KO = O // P  # 2 x chunks
    UNROLL = 8
    assert T % UNROLL == 0

    nc = bacc.Bacc("TRN2", target_bir_lowering=False, debug=False,
                   num_devices=N_CORES)

    # ---- DRAM I/O ----
    zT_d = nc.dram_tensor("zT", [L, B_loc], f32r, kind="ExternalInput")
    WlhT_d = nc.dram_tensor("WlhT", [P, L // P, H], f32r, kind="ExternalInput")
    WihT_d = nc.dram_tensor("WihT", [P, KO, H3], f32r, kind="ExternalInput")
    WhhT_d = nc.dram_tensor("WhhT", [P, KH, H3], f32r, kind="ExternalInput")
    WoutT_d = nc.dram_tensor("WoutT", [P, KH, O], f32r, kind="ExternalInput")
    if has_bias:
        # [b_rz (2H) | b_inn (H) | b_hn (H) | b_out (O) | b_lh (H)] combined row
        bias_d = nc.dram_tensor("bias", [1, 2 * H + H + H + O + H], f32r,
                                kind="ExternalInput")
    y_d = nc.dram_tensor("y", [B_loc, T, O], f32, kind="ExternalOutput")

    with tile.TileContext(nc) as tc:
        # ---- persistent SBUF ----
        WlhT = nc.alloc_sbuf_tensor("WlhT_s", [P, L // P, H], f32r)
        WihT = nc.alloc_sbuf_tensor("WihT_s", [P, KO, H3], f32r)
        WhhT = nc.alloc_sbuf_tensor("WhhT_s", [P, KH, H3], f32r)
        WoutT = nc.alloc_sbuf_tensor("WoutT_s", [P, KH, O], f32r)
        zT = nc.alloc_sbuf_tensor("zT_s", [L, B_loc], f32r)
        hT = nc.alloc_sbuf_tensor("hT_s", [P, KH, B_loc], f32r)  # h.T chunks
        xT = nc.alloc_sbuf_tensor("xT_s", [P, KO, B_loc], f32r)  # x.T chunks
        h_bm = nc.alloc_sbuf_tensor("h_bm_s", [B_loc, H], f32)   # batch-major h
        ybuf = nc.alloc_sbuf_tensor("ybuf_s", [B_loc, UNROLL, O], f32)
        ident = nc.alloc_sbuf_tensor("ident_s", [B_loc, B_loc], f32)
        if has_bias:
            bias_s = nc.alloc_sbuf_tensor("bias_s", [1, 5 * H + O], f32r)
            ones_s = nc.alloc_sbuf_tensor("ones_s", [1, B_loc], f32r)

        for dram, sbuf in [(WlhT_d, WlhT), (WihT_d, WihT), (WhhT_d, WhhT),
                           (WoutT_d, WoutT), (zT_d, zT)]:
            nc.sync.dma_start(sbuf[:], dram[:])
        if has_bias:
            nc.sync.dma_start(bias_s[:], bias_d[:])
            nc.gpsimd.memset(ones_s[:], 1.0)
        make_identity(nc, ident[:])
        zero_sb = nc.alloc_sbuf_tensor("zero_s", [P, KO * B_loc], f32)
        nc.gpsimd.memset(zero_sb[:], 0.0)
        nc.vector.tensor_copy(xT[:], zero_sb[:])
        ones_bm = nc.alloc_sbuf_tensor("ones_bm_s", [B_loc, 512], f32)
        nc.gpsimd.memset(ones_bm[:], 1.0)
        ones_full = nc.alloc_sbuf_tensor("ones_full_s", [B_loc, H], f32)
        nc.gpsimd.memset(ones_full[:], 1.0)

        sb_pool = tc.alloc_tile_pool(name="sb", bufs=2)
        ps_pool = tc.alloc_tile_pool(name="ps", bufs=1, space="PSUM")
        acc_pool = tc.alloc_tile_pool(name="acc", bufs=4, space="PSUM")

        def bias_row(ps_chunk, col0, ncols, start):
            # ps_chunk += ones.T @ bias_row  (adds bias to every batch row)
            nc.tensor.matmul(ps_chunk,
                             ones_s[:, :],
                             bias_s[:, col0:col0 + ncols],
                             start=start, stop=False)

        # ---- h0 = tanh(z @ W_lh.T) ----
        for j in range(H // 512):
            cs = slice(j * 512, (j + 1) * 512)
            h0_ps = acc_pool.tile([B_loc, 512], f32, tag="acc")
            first = True
            if has_bias:
                bias_row(h0_ps[:], 4 * H + O + j * 512, 512, True)
                first = False
            nc.tensor.matmul(h0_ps[:], zT[:, :],
                             WlhT[:, 0, cs],
                             start=first, stop=True)
            nc.scalar.activation(h_bm[:, cs], h0_ps[:],
                                 mybir.ActivationFunctionType.Sigmoid,
                                 scale=2.0)
            nc.vector.scalar_tensor_tensor(
                h_bm[:, cs], h_bm[:, cs], 2.0, ones_bm[:],
                mybir.AluOpType.mult, mybir.AluOpType.subtract)
        tp0 = ps_pool.tile([P, KH * B_loc], f32, tag="tp")
        for c in range(KH):
            nc.tensor.transpose(tp0[:, c * B_loc:(c + 1) * B_loc],
                                h_bm[:, c * P:(c + 1) * P], ident[:])
        nc.vector.tensor_copy(hT[:], tp0[:])

        # ---- the scan ----
        NC512 = H // 512  # 512-col accumulation chunks per gate
        # n-gate chunking: big first, small last (shorter serial tail)
        NCHUNKS = [(0, 512), (512, 256), (768, 256)]
        assert sum(w for _, w in NCHUNKS) == H

        def gate_mm(ps, wcol0, first):
            """accumulate h @ W_hh.T[:, wcol] + x @ W_ih.T[:, wcol] into ps.
            h-part first: x arrives late (feedback through out), so keep the
            x matmuls last in the accumulation group."""
            wcs = slice(wcol0, wcol0 + 512)
            if has_bias:
                bias_row(ps, wcol0, 512, True)
                first = False
            for k in range(KH):
                nc.tensor.matmul(ps, hT[:, k, :],
                                 WhhT[:, k, wcs],
                                 start=first and k == 0, stop=False)
            for k in range(KO):
                nc.tensor.matmul(ps, xT[:, k, :],
                                 WihT[:, k, wcs],
                                 start=False, stop=k == KO - 1)

        def step(j, iv0):
            r_sb = sb_pool.tile([B_loc, H], f32, tag="r")
            z_sb = sb_pool.tile([B_loc, H], f32, tag="z")
            # r and z gates: gi+gh accumulated together, drained eagerly
            for g, gsb in ((0, r_sb), (1, z_sb)):
                for c in range(NC512):
                    cs = slice(c * 512, (c + 1) * 512)
                    ps = acc_pool.tile([B_loc, 512], f32, tag="acc")
                    gate_mm(ps[:], g * H + c * 512, True)
                    if "sig" not in ablate:
                        nc.scalar.activation(gsb[:, cs], ps[:],
                                             mybir.ActivationFunctionType.Sigmoid)
            # zh = z*h and omz = 1-z: off the critical n-chain, computed
            # as soon as z is ready
            if "math" not in ablate:
                zh_sb = sb_pool.tile([B_loc, H], f32, tag="zh")
                nc.vector.tensor_mul(zh_sb[:], z_sb[:], h_bm[:])
                omz_sb = sb_pool.tile([B_loc, H], f32, tag="omz")
                nc.vector.scalar_tensor_tensor(
                    omz_sb[:], z_sb[:], -1.0, ones_full[:],
                    mybir.AluOpType.mult, mybir.AluOpType.add)
            # n gate per chunk: i_n (x-part, evicted to SBUF by ACT while the
            # h-part accumulates), then h_n; n = tanh(i_n + r*h_n) via
            # 2*sigmoid(2.) - 1 (sigmoid-only ACT table); h' = n*omz + zh
            for (c0, cw) in NCHUNKS:
                cs = slice(c0, c0 + cw)
                wcs = slice(2 * H + c0, 2 * H + c0 + cw)
                inn = acc_pool.tile([B_loc, 512], f32, tag="acc")
                first = True
                if has_bias:
                    bias_row(inn[:, :cw], 2 * H + c0, cw, True)
                    first = False
                for k in range(KO):
                    nc.tensor.matmul(inn[:, :cw], xT[:, k, :],
                                     WihT[:, k, wcs],
                                     start=first and k == 0, stop=k == KO - 1)
                inn_sb = sb_pool.tile([B_loc, 512], f32, tag="inn_sb")
                if "math" not in ablate:
                    nc.scalar.activation(inn_sb[:, :cw], inn[:, :cw],
                                         mybir.ActivationFunctionType.Copy)
                hn = acc_pool.tile([B_loc, 512], f32, tag="acc")
                first = True
                if has_bias:
                    bias_row(hn[:, :cw], 3 * H + c0, cw, True)
                    first = False
                for k in range(KH):
                    nc.tensor.matmul(hn[:, :cw], hT[:, k, :],
                                     WhhT[:, k, wcs],
                                     start=first and k == 0, stop=k == KH - 1)
                if "math" in ablate:
                    continue
                t0 = sb_pool.tile([B_loc, 512], f32, tag="t0")
                nc.vector.tensor_mul(t0[:, :cw], r_sb[:, cs], hn[:, :cw])
                npre = sb_pool.tile([B_loc, 512], f32, tag="npre")
                nc.vector.tensor_add(npre[:, :cw], t0[:, :cw], inn_sb[:, :cw])
                s_sb = sb_pool.tile([B_loc, 512], f32, tag="n")
                nc.scalar.activation(s_sb[:, :cw], npre[:, :cw],
                                     mybir.ActivationFunctionType.Sigmoid,
                                     scale=2.0)
                n_sb = sb_pool.tile([B_loc, 512], f32, tag="n2")
                nc.vector.scalar_tensor_tensor(
                    n_sb[:, :cw], s_sb[:, :cw], 2.0, ones_bm[:, :cw],
                    mybir.AluOpType.mult, mybir.AluOpType.subtract)
                u_sb = sb_pool.tile([B_loc, 512], f32, tag="u")
                nc.vector.tensor_mul(u_sb[:, :cw], n_sb[:, :cw], omz_sb[:, cs])
                nc.vector.tensor_add(h_bm[:, cs], u_sb[:, :cw], zh_sb[:, cs])

            if "tp" not in ablate:
                tp = ps_pool.tile([P, KH * B_loc], f32, tag="tp")
                for cc in range(KH):
                    nc.tensor.transpose(tp[:, cc * B_loc:(cc + 1) * B_loc],
                                        h_bm[:, cc * P:(cc + 1) * P], ident[:])
                nc.vector.tensor_copy(hT[:], tp[:])

            # out = h' @ W_out.T  (also next x)
            if "out" in ablate:
                return
            out_ps = ps_pool.tile([B_loc, O], f32, tag="out")
            first = True
            if has_bias:
                bias_row(out_ps[:, :], 4 * H, O, True)
                first = False
            for k in range(KH):
                nc.tensor.matmul(out_ps[:], hT[:, k, :],
                                 WoutT[:, k, :],
                                 start=first and k == 0, stop=k == KH - 1)
            nc.vector.tensor_copy(ybuf[:, j, :], out_ps[:])
            # x.T for next step
            xp = ps_pool.tile([P, KO * B_loc], f32, tag="xp")
            for c in range(KO):
                nc.tensor.transpose(xp[:, c * B_loc:(c + 1) * B_loc],
                                    ybuf[:, j, c * P:(c + 1) * P], ident[:])
            nc.vector.tensor_copy(xT[:], xp[:])

        def block(iv0, unroll):
            assert unroll == UNROLL
            for j in range(UNROLL):
                step(j, iv0)
            nc.sync.dma_start(y_d[:, bass.ds(iv0, UNROLL), :], ybuf[:])

        if static:
            for b in range(T // UNROLL):
                block(b * UNROLL, UNROLL)
        else:
            tc.For_i_unrolled_general(
                start=0, end=T, step=1,
                unrollable_body=block, max_unroll=UNROLL,
                hint_engines=(mybir.EngineType.PE,),
            )
        acc_pool.release()
        ps_pool.release()
        sb_pool.release()

    nc.compile()
    return nc


_CACHE = {}


def _round_f32r(a):
    """Round-to-nearest-even to fp32r (11 mantissa bits kept)."""
    b = np.ascontiguousarray(a, dtype=np.float32).view(np.uint32)
    lsb = (b >> np.uint32(12)) & np.uint32(1)
    out = (b + np.uint32(0x7FF) + lsb) & np.uint32(0xFFFFF000)
    return out.view(np.float32)


def _prepare(z, W_lh, b_lh, W_ih, b_ih, W_hh, b_hh, W_out, b_out, seq_len):
    z = np.asarray(z, dtype=np.float32)
    W_lh = np.asarray(W_lh, dtype=np.float32)
    W_ih = np.asarray(W_ih, dtype=np.float32)
    W_hh = np.asarray(W_hh, dtype=np.float32)
    W_out = np.asarray(W_out, dtype=np.float32)
    b_lh = np.asarray(b_lh, dtype=np.float32)
    b_ih = np.asarray(b_ih, dtype=np.float32)
    b_hh = np.asarray(b_hh, dtype=np.float32)
    b_out = np.asarray(b_out, dtype=np.float32)
    T = int(seq_len)
    B, L = z.shape
    H = W_hh.shape[1]
    O = W_out.shape[0]
    H3 = 3 * H
    B_loc = B // N_CORES
    has_bias = any(np.any(b) for b in (b_lh, b_ih, b_hh, b_out))

    key = (T, B_loc, L, H, O, has_bias)
    if key not in _CACHE:
        _CACHE[key] = _build(*key)
    nc = _CACHE[key]

    def chunked(WT, k):  # [KD*P, N] -> [P, KD, N]
        KD = WT.shape[0] // P
        return np.ascontiguousarray(
            WT.reshape(KD, P, WT.shape[1]).transpose(1, 0, 2))

    common = {
        "WlhT": _round_f32r(chunked(W_lh.T, L // P)),
        "WihT": _round_f32r(chunked(W_ih.T, 2)),
        "WhhT": _round_f32r(chunked(W_hh.T, H // P)),
        "WoutT": _round_f32r(chunked(W_out.T, H // P)),
    }
    if has_bias:
        brz = (b_ih + b_hh)[:2 * H]
        common["bias"] = np.concatenate(
            [brz, b_ih[2 * H:], b_hh[2 * H:], b_out, b_lh]
        ).reshape(1, -1).astype(np.float32)

    in_maps = []
    for c in range(N_CORES):
        m = dict(common)
        m["zT"] = _round_f32r(z[c * B_loc:(c + 1) * B_loc].T)
        in_maps.append(m)
    return nc, in_maps


def kernel(**inputs):
    from concourse import bass_utils

    nc, in_maps = _prepare(**inputs)
    res = bass_utils.run_bass_kernel_spmd(
        nc, in_maps, core_ids=list(range(N_CORES)))
    y = np.concatenate([r["y"] for r in res.results], axis=0)
    return y


def run_traced(inputs):
    from concourse import bass_utils

    nc, in_maps = _prepare(**inputs)
    return bass_utils.run_bass_kernel_spmd(
        nc, in_maps, core_ids=list(range(N_CORES)), trace=True)


# revision 27
# speedup vs baseline: 313.4543x; 307.2977x over previous
"""GRU decoder kernel for trn2, 8-core data-parallel over batch.

Reference semantics (per step, torch-GRU style):
    gi = x @ W_ih.T + b_ih ; gh = h @ W_hh.T + b_hh
    r = sigmoid(gi_r + gh_r); z = sigmoid(gi_z + gh_z)
    n = tanh(gi_n + r * gh_n)
    h' = (1-z)*n + z*h ; out = h' @ W_out.T + b_out  (out feeds back as next x)

Design notes:
 - batch sharded 8 ways (32 rows/core), weights replicated per core in SBUF.
 - batch-major matmuls: lhsT = x.T/h.T chunks (stationary), rhs = weight
   matrices streamed (moving).  float32r dtype -> 1 cycle/row on PE.
 - r,z gates accumulate gi+gh directly in PSUM; n-gate parts kept separate.
 - h' and out are transposed back to feature-major via PE transpose for the
   next step's stationary operands.
"""

import sys

sys.path.insert(0, "/opt/trn_rl_repo")
sys.path.insert(0, "/opt/trn_rl_repo/concourse")

import numpy as np

N_CORES = 8
P = 128  # partitions


def _build(T, B_loc, L, H, O, has_bias, static=False, ablate=(), t_alloc=None, repeat=1, wdt_name='bf16', unroll=16):
    import concourse.bacc as bacc
    import concourse.bass as bass
    import concourse.mybir as mybir
    import concourse.tile as tile
    from concourse.masks import make_identity

    f32 = mybir.dt.float32
    f32r = mybir.dt.float32r
    wdt = {"f32r": f32r, "bf16": mybir.dt.bfloat16}[wdt_name]
    H3 = 3 * H
    KH = H // P  # 8 h chunks
    KO = O // P  # 2 x chunks
    UNROLL = unroll
    assert T % UNROLL == 0

    nc = bacc.Bacc("TRN2", target_bir_lowering=False, debug=False,
                   num_devices=N_CORES)

    # ---- DRAM I/O ----
    zT_d = nc.dram_tensor("zT", [L, B_loc], wdt, kind="ExternalInput")
    WlhT_d = nc.dram_tensor("WlhT", [P, L // P, H], wdt, kind="ExternalInput")
    WihT_d = nc.dram_tensor("WihT", [P, KO, H3], wdt, kind="ExternalInput")
    WhhT_d = nc.dram_tensor("WhhT", [P, KH, H3], wdt, kind="ExternalInput")
    WoutT_d = nc.dram_tensor("WoutT", [P, KH, O], wdt, kind="ExternalInput")
    if has_bias:
        # [b_rz (2H) | b_inn (H) | b_hn (H) | b_out (O) | b_lh (H)] combined row
        bias_d = nc.dram_tensor("bias", [1, 2 * H + H + H + O + H], wdt,
                                kind="ExternalInput")
    y_d = nc.dram_tensor("y", [B_loc, t_alloc or T, O], f32, kind="ExternalOutput")

    with tile.TileContext(nc) as tc:
        # ---- persistent SBUF ----
        WlhT = nc.alloc_sbuf_tensor("WlhT_s", [P, L // P, H], wdt)
        WihT = nc.alloc_sbuf_tensor("WihT_s", [P, KO, H3], wdt)
        WhhT = nc.alloc_sbuf_tensor("WhhT_s", [P, KH, H3], wdt)
        WoutT = nc.alloc_sbuf_tensor("WoutT_s", [P, KH, O], wdt)
        zT = nc.alloc_sbuf_tensor("zT_s", [L, B_loc], wdt)
        hT = nc.alloc_sbuf_tensor("hT_s", [P, KH, B_loc], wdt)  # h.T chunks
        xT = nc.alloc_sbuf_tensor("xT_s", [P, KO, B_loc], wdt)  # x.T chunks
        h_bm = nc.alloc_sbuf_tensor("h_bm_s", [B_loc, H], f32)   # batch-major h
        ybuf = nc.alloc_sbuf_tensor("ybuf_s", [B_loc, UNROLL, O], f32)
        ident = nc.alloc_sbuf_tensor("ident_s", [B_loc, B_loc], f32)
        if has_bias:
            bias_s = nc.alloc_sbuf_tensor("bias_s", [1, 5 * H + O], wdt)
            ones_s = nc.alloc_sbuf_tensor("ones_s", [1, B_loc], wdt)

        for dram, sbuf in [(WlhT_d, WlhT), (WihT_d, WihT), (WhhT_d, WhhT),
                           (WoutT_d, WoutT), (zT_d, zT)]:
            nc.sync.dma_start(sbuf[:], dram[:])
        if has_bias:
            nc.sync.dma_start(bias_s[:], bias_d[:])
            nc.gpsimd.memset(ones_s[:], 1.0)
        make_identity(nc, ident[:])
        zero_sb = nc.alloc_sbuf_tensor("zero_s", [P, KO * B_loc], f32)
        nc.gpsimd.memset(zero_sb[:], 0.0)
        nc.vector.tensor_copy(xT[:], zero_sb[:])
        ones_bm = nc.alloc_sbuf_tensor("ones_bm_s", [B_loc, 512], f32)
        nc.gpsimd.memset(ones_bm[:], 1.0)
        ones_full = nc.alloc_sbuf_tensor("ones_full_s", [B_loc, H], f32)
        nc.gpsimd.memset(ones_full[:], 1.0)

        sb_pool = tc.alloc_tile_pool(name="sb", bufs=2)
        ps_pool = tc.alloc_tile_pool(name="ps", bufs=1, space="PSUM")
        acc_pool = tc.alloc_tile_pool(name="acc", bufs=5, space="PSUM")

        def bias_row(ps_chunk, col0, ncols, start):
            # ps_chunk += ones.T @ bias_row  (adds bias to every batch row)
            nc.tensor.matmul(ps_chunk,
                             ones_s[:, :],
                             bias_s[:, col0:col0 + ncols],
                             start=start, stop=False)

        # ---- h0 = tanh(z @ W_lh.T) ----
        for j in range(H // 512):
            cs = slice(j * 512, (j + 1) * 512)
            h0_ps = acc_pool.tile([B_loc, 512], f32, tag="acc")
            first = True
            if has_bias:
                bias_row(h0_ps[:], 4 * H + O + j * 512, 512, True)
                first = False
            nc.tensor.matmul(h0_ps[:], zT[:, :],
                             WlhT[:, 0, cs],
                             start=first, stop=True)
            nc.scalar.activation(h_bm[:, cs], h0_ps[:],
                                 mybir.ActivationFunctionType.Sigmoid,
                                 scale=2.0)
            nc.vector.scalar_tensor_tensor(
                h_bm[:, cs], h_bm[:, cs], 2.0, ones_bm[:],
                mybir.AluOpType.mult, mybir.AluOpType.subtract)
        tp0 = ps_pool.tile([P, KH * B_loc], f32, tag="tp")
        for c in range(KH):
            nc.tensor.transpose(tp0[:, c * B_loc:(c + 1) * B_loc],
                                h_bm[:, c * P:(c + 1) * P], ident[:])
        nc.vector.tensor_copy(hT[:], tp0[:])

        # ---- the scan ----
        NC512 = H // 512  # 512-col accumulation chunks per gate
        # n-gate chunking: big first, small last (shorter serial tail)
        NCHUNKS = [(0, 512), (512, 256), (768, 256)]
        assert sum(w for _, w in NCHUNKS) == H

        def gate_mm(ps, wcol0, first):
            """accumulate h @ W_hh.T[:, wcol] + x @ W_ih.T[:, wcol] into ps.
            h-part first: x arrives late (feedback through out), so keep the
            x matmuls last in the accumulation group."""
            wcs = slice(wcol0, wcol0 + 512)
            if has_bias:
                bias_row(ps, wcol0, 512, True)
                first = False
            for k in range(KH):
                nc.tensor.matmul(ps, hT[:, k, :],
                                 WhhT[:, k, wcs],
                                 start=first and k == 0, stop=False)
            for k in range(KO):
                nc.tensor.matmul(ps, xT[:, k, :],
                                 WihT[:, k, wcs],
                                 start=False, stop=k == KO - 1)

        def step(j, iv0):
            r_sb = sb_pool.tile([B_loc, H], f32, tag="r")
            z_sb = sb_pool.tile([B_loc, H], f32, tag="z")
            # r and z gates: gi+gh accumulated together, drained eagerly
            for g, gsb in ((0, r_sb), (1, z_sb)):
                for c in range(NC512):
                    cs = slice(c * 512, (c + 1) * 512)
                    ps = acc_pool.tile([B_loc, 512], f32, tag="acc")
                    gate_mm(ps[:], g * H + c * 512, True)
                    if "sig" not in ablate:
                        nc.scalar.activation(gsb[:, cs], ps[:],
                                             mybir.ActivationFunctionType.Sigmoid)
            # zh = z*h and omz = 1-z: off the critical n-chain, computed
            # as soon as z is ready
            if "math" not in ablate:
                zh_sb = sb_pool.tile([B_loc, H], f32, tag="zh")
                nc.vector.tensor_mul(zh_sb[:], z_sb[:], h_bm[:])
                omz_sb = sb_pool.tile([B_loc, H], f32, tag="omz")
                nc.vector.scalar_tensor_tensor(
                    omz_sb[:], z_sb[:], -1.0, ones_full[:],
                    mybir.AluOpType.mult, mybir.AluOpType.add)
            # n gate per chunk: i_n (x-part, evicted to SBUF by ACT while the
            # h-part accumulates), then h_n; n = tanh(i_n + r*h_n) via
            # 2*sigmoid(2.) - 1 (sigmoid-only ACT table); h' = n*omz + zh
            for (c0, cw) in NCHUNKS:
                cs = slice(c0, c0 + cw)
                wcs = slice(2 * H + c0, 2 * H + c0 + cw)
                inn = acc_pool.tile([B_loc, 512], f32, tag="acc")
                first = True
                if has_bias:
                    bias_row(inn[:, :cw], 2 * H + c0, cw, True)
                    first = False
                for k in range(KO):
                    nc.tensor.matmul(inn[:, :cw], xT[:, k, :],
                                     WihT[:, k, wcs],
                                     start=first and k == 0, stop=k == KO - 1)
                inn_sb = sb_pool.tile([B_loc, 512], f32, tag="inn_sb")
                if "math" not in ablate:
                    nc.scalar.activation(inn_sb[:, :cw], inn[:, :cw],
                                         mybir.ActivationFunctionType.Copy)
                hn = acc_pool.tile([B_loc, 512], f32, tag="acc")
                first = True
                if has_bias:
                    bias_row(hn[:, :cw], 3 * H + c0, cw, True)
                    first = False
                for k in range(KH):
                    nc.tensor.matmul(hn[:, :cw], hT[:, k, :],
                                     WhhT[:, k, wcs],
                                     start=first and k == 0, stop=k == KH - 1)
                if "math" in ablate:
                    continue
                t0 = sb_pool.tile([B_loc, 512], f32, tag="t0")
                nc.vector.tensor_mul(t0[:, :cw], r_sb[:, cs], hn[:, :cw])
                npre = sb_pool.tile([B_loc, 512], f32, tag="npre")
                nc.vector.tensor_add(npre[:, :cw], t0[:, :cw], inn_sb[:, :cw])
                s_sb = sb_pool.tile([B_loc, 512], f32, tag="n")
                nc.scalar.activation(s_sb[:, :cw], npre[:, :cw],
                                     mybir.ActivationFunctionType.Sigmoid,
                                     scale=2.0)
                n_sb = sb_pool.tile([B_loc, 512], f32, tag="n2")
                nc.vector.scalar_tensor_tensor(
                    n_sb[:, :cw], s_sb[:, :cw], 2.0, ones_bm[:, :cw],
                    mybir.AluOpType.mult, mybir.AluOpType.subtract)
                u_sb = sb_pool.tile([B_loc, 512], f32, tag="u")
                nc.vector.tensor_mul(u_sb[:, :cw], n_sb[:, :cw], omz_sb[:, cs])
                nc.vector.tensor_add(h_bm[:, cs], u_sb[:, :cw], zh_sb[:, cs])

            if "tp" not in ablate:
                tp = ps_pool.tile([P, KH * B_loc], f32, tag="tp")
                for cc in range(KH):
                    nc.tensor.transpose(tp[:, cc * B_loc:(cc + 1) * B_loc],
                                        h_bm[:, cc * P:(cc + 1) * P], ident[:])
                nc.vector.tensor_copy(hT[:], tp[:])

            # out = h' @ W_out.T  (also next x)
            if "out" in ablate:
                return
            out_ps = ps_pool.tile([B_loc, O], f32, tag="out")
            first = True
            if has_bias:
                bias_row(out_ps[:, :], 4 * H, O, True)
                first = False
            for k in range(KH):
                nc.tensor.matmul(out_ps[:], hT[:, k, :],
                                 WoutT[:, k, :],
                                 start=first and k == 0, stop=k == KH - 1)
            nc.vector.tensor_copy(ybuf[:, j, :], out_ps[:])
            # x.T for next step
            xp = ps_pool.tile([P, KO * B_loc], f32, tag="xp")
            for c in range(KO):
                nc.tensor.transpose(xp[:, c * B_loc:(c + 1) * B_loc],
                                    ybuf[:, j, c * P:(c + 1) * P], ident[:])
            nc.vector.tensor_copy(xT[:], xp[:])

        def block(iv0, unroll):
            assert unroll == UNROLL
            for j in range(UNROLL):
                step(j, iv0)
            nc.sync.dma_start(y_d[:, bass.ds(iv0, UNROLL), :], ybuf[:])

        if static:
            for b in range(T // UNROLL):
                block(b * UNROLL, UNROLL)
        elif repeat > 1:
            # timing mode: amplify the scan; y written at fixed offset 0
            def block0(iv0, unroll):
                for j in range(UNROLL):
                    step(j, 0)
                nc.sync.dma_start(y_d[:, 0:UNROLL, :], ybuf[:])
            tc.For_i_unrolled_general(
                start=0, end=T * repeat, step=1,
                unrollable_body=block0, max_unroll=UNROLL,
                hint_engines=(mybir.EngineType.PE,),
            )
        else:
            tc.For_i_unrolled_general(
                start=0, end=T, step=1,
                unrollable_body=block, max_unroll=UNROLL,
                hint_engines=(mybir.EngineType.PE,),
            )
        acc_pool.release()
        ps_pool.release()
        sb_pool.release()

    nc.compile()
    return nc


_CACHE = {}


def _round_f32r(a):
    """Round-to-nearest-even to fp32r (11 mantissa bits kept)."""
    b = np.ascontiguousarray(a, dtype=np.float32).view(np.uint32)
    lsb = (b >> np.uint32(12)) & np.uint32(1)
    out = (b + np.uint32(0x7FF) + lsb) & np.uint32(0xFFFFF000)
    return out.view(np.float32)


def _prepare(z, W_lh, b_lh, W_ih, b_ih, W_hh, b_hh, W_out, b_out, seq_len):
    z = np.asarray(z, dtype=np.float32)
    W_lh = np.asarray(W_lh, dtype=np.float32)
    W_ih = np.asarray(W_ih, dtype=np.float32)
    W_hh = np.asarray(W_hh, dtype=np.float32)
    W_out = np.asarray(W_out, dtype=np.float32)
    b_lh = np.asarray(b_lh, dtype=np.float32)
    b_ih = np.asarray(b_ih, dtype=np.float32)
    b_hh = np.asarray(b_hh, dtype=np.float32)
    b_out = np.asarray(b_out, dtype=np.float32)
    T = int(seq_len)
    B, L = z.shape
    H = W_hh.shape[1]
    O = W_out.shape[0]
    H3 = 3 * H
    B_loc = B // N_CORES
    has_bias = any(np.any(b) for b in (b_lh, b_ih, b_hh, b_out))

    key = (T, B_loc, L, H, O, has_bias)
    if key not in _CACHE:
        _CACHE[key] = _build(*key)
    nc = _CACHE[key]
    import ml_dtypes
    bf16 = ml_dtypes.bfloat16

    def chunked(WT, k):  # [KD*P, N] -> [P, KD, N]
        KD = WT.shape[0] // P
        return np.ascontiguousarray(
            WT.reshape(KD, P, WT.shape[1]).transpose(1, 0, 2))

    common = {
        "WlhT": chunked(W_lh.T, L // P).astype(bf16),
        "WihT": chunked(W_ih.T, 2).astype(bf16),
        "WhhT": chunked(W_hh.T, H // P).astype(bf16),
        "WoutT": chunked(W_out.T, H // P).astype(bf16),
    }
    if has_bias:
        brz = (b_ih + b_hh)[:2 * H]
        common["bias"] = np.concatenate(
            [brz, b_ih[2 * H:], b_hh[2 * H:], b_out, b_lh]
        ).reshape(1, -1).astype(bf16)

    in_maps = []
    for c in range(N_CORES):
        m = dict(common)
        m["zT"] = np.ascontiguousarray(z[c * B_loc:(c + 1) * B_loc].T).astype(bf16)
        in_maps.append(m)
    return nc, in_maps


def kernel(**inputs):
    from concourse import bass_utils

    nc, in_maps = _prepare(**inputs)
    res = bass_utils.run_bass_kernel_spmd(
        nc, in_maps, core_ids=list(range(N_CORES)))
    y = np.concatenate([r["y"] for r in res.results], axis=0)
    return y


def run_traced(inputs):
    from concourse import bass_utils

    nc, in_maps = _prepare(**inputs)
    return bass_utils.run_bass_kernel_spmd(
        nc, in_maps, core_ids=list(range(N_CORES)), trace=True)


# revision 31
# speedup vs baseline: 383.4319x; 1.2232x over previous
"""GRU decoder kernel for trn2, 8-core data-parallel over batch.

Reference semantics (per step, torch-GRU style):
    gi = x @ W_ih.T + b_ih ; gh = h @ W_hh.T + b_hh
    r = sigmoid(gi_r + gh_r); z = sigmoid(gi_z + gh_z)
    n = tanh(gi_n + r * gh_n)
    h' = (1-z)*n + z*h ; out = h' @ W_out.T + b_out  (out feeds back as next x)

Design notes:
 - batch sharded 8 ways (32 rows/core), weights replicated per core in SBUF.
 - batch-major matmuls: lhsT = x.T/h.T chunks (stationary), rhs = weight
   matrices streamed (moving).  float32r dtype -> 1 cycle/row on PE.
 - r,z gates accumulate gi+gh directly in PSUM; n-gate parts kept separate.
 - h' and out are transposed back to feature-major via PE transpose for the
   next step's stationary operands.
"""

import sys

sys.path.insert(0, "/opt/trn_rl_repo")
sys.path.insert(0, "/opt/trn_rl_repo/concourse")

import numpy as np

N_CORES = 8
P = 128  # partitions


def _build(T, B_loc, L, H, O, has_bias, static=False, ablate=(), t_alloc=None, repeat=1, wdt_name='bf16', unroll=16):
    import concourse.bacc as bacc
    import concourse.bass as bass
    import concourse.mybir as mybir
    import concourse.tile as tile
    from concourse.masks import make_identity

    f32 = mybir.dt.float32
    f32r = mybir.dt.float32r
    wdt = {"f32r": f32r, "bf16": mybir.dt.bfloat16}[wdt_name]
    H3 = 3 * H
    KH = H // P  # 8 h chunks
    KO = O // P  # 2 x chunks
    UNROLL = unroll
    assert T % UNROLL == 0

    nc = bacc.Bacc("TRN2", target_bir_lowering=False, debug=False,
                   num_devices=N_CORES)

    # ---- DRAM I/O ----
    zT_d = nc.dram_tensor("zT", [L, B_loc], wdt, kind="ExternalInput")
    WlhT_d = nc.dram_tensor("WlhT", [P, L // P, H], wdt, kind="ExternalInput")
    WihT_d = nc.dram_tensor("WihT", [P, KO, H3], wdt, kind="ExternalInput")
    WhhT_d = nc.dram_tensor("WhhT", [P, KH, H3], wdt, kind="ExternalInput")
    WoutT_d = nc.dram_tensor("WoutT", [P, KH, O], wdt, kind="ExternalInput")
    if has_bias:
        # [b_rz (2H) | b_inn (H) | b_hn (H) | b_out (O) | b_lh (H)] combined row
        bias_d = nc.dram_tensor("bias", [1, 2 * H + H + H + O + H], wdt,
                                kind="ExternalInput")
    y_d = nc.dram_tensor("y", [B_loc, t_alloc or T, O], f32, kind="ExternalOutput")

    with tile.TileContext(nc) as tc:
        # ---- persistent SBUF ----
        WlhT = nc.alloc_sbuf_tensor("WlhT_s", [P, L // P, H], wdt)
        WihT = nc.alloc_sbuf_tensor("WihT_s", [P, KO, H3], wdt)
        WhhT = nc.alloc_sbuf_tensor("WhhT_s", [P, KH, H3], wdt)
        WoutT = nc.alloc_sbuf_tensor("WoutT_s", [P, KH, O], wdt)
        zT = nc.alloc_sbuf_tensor("zT_s", [L, B_loc], wdt)
        hT = nc.alloc_sbuf_tensor("hT_s", [P, KH, B_loc], wdt)  # h.T chunks
        xT = nc.alloc_sbuf_tensor("xT_s", [P, KO, B_loc], wdt)  # x.T chunks
        h_bm = nc.alloc_sbuf_tensor("h_bm_s", [B_loc, H], f32)   # batch-major h
        ybuf = nc.alloc_sbuf_tensor("ybuf_s", [B_loc, UNROLL, O], f32)
        ident = nc.alloc_sbuf_tensor("ident_s", [B_loc, B_loc], f32)
        if has_bias:
            bias_s = nc.alloc_sbuf_tensor("bias_s", [1, 5 * H + O], wdt)
            ones_s = nc.alloc_sbuf_tensor("ones_s", [1, B_loc], wdt)

        for dram, sbuf in [(WlhT_d, WlhT), (WihT_d, WihT), (WhhT_d, WhhT),
                           (WoutT_d, WoutT), (zT_d, zT)]:
            nc.sync.dma_start(sbuf[:], dram[:])
        if has_bias:
            nc.sync.dma_start(bias_s[:], bias_d[:])
            nc.gpsimd.memset(ones_s[:], 1.0)
        make_identity(nc, ident[:])
        zero_sb = nc.alloc_sbuf_tensor("zero_s", [P, KO * B_loc], f32)
        nc.gpsimd.memset(zero_sb[:], 0.0)
        nc.vector.tensor_copy(xT[:], zero_sb[:])
        ones_bm = nc.alloc_sbuf_tensor("ones_bm_s", [B_loc, 512], f32)
        nc.gpsimd.memset(ones_bm[:], 1.0)
        ones_full = nc.alloc_sbuf_tensor("ones_full_s", [B_loc, H], f32)
        nc.gpsimd.memset(ones_full[:], 1.0)

        sb_pool = tc.alloc_tile_pool(name="sb", bufs=2)
        ps_pool = tc.alloc_tile_pool(name="ps", bufs=1, space="PSUM")
        acc_pool = tc.alloc_tile_pool(name="acc", bufs=5, space="PSUM")

        def bias_row(ps_chunk, col0, ncols, start):
            # ps_chunk += ones.T @ bias_row  (adds bias to every batch row)
            nc.tensor.matmul(ps_chunk,
                             ones_s[:, :],
                             bias_s[:, col0:col0 + ncols],
                             start=start, stop=False)

        # ---- h0 = tanh(z @ W_lh.T) ----
        for j in range(H // 512):
            cs = slice(j * 512, (j + 1) * 512)
            h0_ps = acc_pool.tile([B_loc, 512], f32, tag="acc")
            first = True
            if has_bias:
                bias_row(h0_ps[:], 4 * H + O + j * 512, 512, True)
                first = False
            nc.tensor.matmul(h0_ps[:], zT[:, :],
                             WlhT[:, 0, cs],
                             start=first, stop=True)
            nc.scalar.activation(h_bm[:, cs], h0_ps[:],
                                 mybir.ActivationFunctionType.Sigmoid,
                                 scale=2.0)
            nc.vector.scalar_tensor_tensor(
                h_bm[:, cs], h_bm[:, cs], 2.0, ones_bm[:],
                mybir.AluOpType.mult, mybir.AluOpType.subtract)
        tp0 = ps_pool.tile([P, KH * B_loc], f32, tag="tp")
        for c in range(KH):
            nc.tensor.transpose(tp0[:, c * B_loc:(c + 1) * B_loc],
                                h_bm[:, c * P:(c + 1) * P], ident[:])
        nc.vector.tensor_copy(hT[:], tp0[:])

        # ---- the scan ----
        NC512 = H // 512  # 512-col accumulation chunks per gate
        # n-gate chunking: big first, small last (shorter serial tail)
        NCHUNKS = [(0, 512), (512, 256), (768, 256)]
        assert sum(w for _, w in NCHUNKS) == H

        def gate_mm(ps, wcol0, first):
            """accumulate h @ W_hh.T[:, wcol] + x @ W_ih.T[:, wcol] into ps.
            h-part first: x arrives late (feedback through out), so keep the
            x matmuls last in the accumulation group."""
            wcs = slice(wcol0, wcol0 + 512)
            if has_bias:
                bias_row(ps, wcol0, 512, True)
                first = False
            for k in range(KH):
                nc.tensor.matmul(ps, hT[:, k, :],
                                 WhhT[:, k, wcs],
                                 start=first and k == 0, stop=False)
            for k in range(KO):
                nc.tensor.matmul(ps, xT[:, k, :],
                                 WihT[:, k, wcs],
                                 start=False, stop=k == KO - 1)

        def step(j, iv0):
            r_sb = sb_pool.tile([B_loc, H], f32, tag="r")
            z_sb = sb_pool.tile([B_loc, H], f32, tag="z")
            # r and z gates: gi+gh accumulated together, drained eagerly
            for g, gsb in ((0, r_sb), (1, z_sb)):
                for c in range(NC512):
                    cs = slice(c * 512, (c + 1) * 512)
                    ps = acc_pool.tile([B_loc, 512], f32, tag="acc")
                    gate_mm(ps[:], g * H + c * 512, True)
                    if "sig" not in ablate:
                        nc.scalar.activation(gsb[:, cs], ps[:],
                                             mybir.ActivationFunctionType.Sigmoid)
            # zh = z*h and omz = 1-z: off the critical n-chain, computed
            # as soon as z is ready
            if "math" not in ablate:
                zh_sb = sb_pool.tile([B_loc, H], f32, tag="zh")
                nc.vector.tensor_mul(zh_sb[:], z_sb[:], h_bm[:])
                omz_sb = sb_pool.tile([B_loc, H], f32, tag="omz")
                nc.vector.scalar_tensor_tensor(
                    omz_sb[:], z_sb[:], -1.0, ones_full[:],
                    mybir.AluOpType.mult, mybir.AluOpType.add)
            # n gate per chunk: i_n (x-part, evicted to SBUF by ACT while the
            # h-part accumulates), then h_n; n = tanh(i_n + r*h_n) via
            # 2*sigmoid(2.) - 1 (sigmoid-only ACT table); h' = n*omz + zh
            for (c0, cw) in NCHUNKS:
                cs = slice(c0, c0 + cw)
                wcs = slice(2 * H + c0, 2 * H + c0 + cw)
                inn = acc_pool.tile([B_loc, 512], f32, tag="acc")
                first = True
                if has_bias:
                    bias_row(inn[:, :cw], 2 * H + c0, cw, True)
                    first = False
                for k in range(KO):
                    nc.tensor.matmul(inn[:, :cw], xT[:, k, :],
                                     WihT[:, k, wcs],
                                     start=first and k == 0, stop=k == KO - 1)
                inn_sb = sb_pool.tile([B_loc, 512], f32, tag="inn_sb")
                if "math" not in ablate:
                    nc.scalar.activation(inn_sb[:, :cw], inn[:, :cw],
                                         mybir.ActivationFunctionType.Copy)
                hn = acc_pool.tile([B_loc, 512], f32, tag="acc")
                first = True
                if has_bias:
                    bias_row(hn[:, :cw], 3 * H + c0, cw, True)
                    first = False
                for k in range(KH):
                    nc.tensor.matmul(hn[:, :cw], hT[:, k, :],
                                     WhhT[:, k, wcs],
                                     start=first and k == 0, stop=k == KH - 1)
                if "math" in ablate:
                    continue
                t0 = sb_pool.tile([B_loc, 512], f32, tag="t0")
                nc.vector.tensor_mul(t0[:, :cw], r_sb[:, cs], hn[:, :cw])
                npre = sb_pool.tile([B_loc, 512], f32, tag="npre")
                nc.vector.tensor_add(npre[:, :cw], t0[:, :cw], inn_sb[:, :cw])
                s_sb = sb_pool.tile([B_loc, 512], f32, tag="n")
                nc.scalar.activation(s_sb[:, :cw], npre[:, :cw],
                                     mybir.ActivationFunctionType.Sigmoid,
                                     scale=2.0)
                n_sb = sb_pool.tile([B_loc, 512], f32, tag="n2")
                nc.vector.scalar_tensor_tensor(
                    n_sb[:, :cw], s_sb[:, :cw], 2.0, ones_bm[:, :cw],
                    mybir.AluOpType.mult, mybir.AluOpType.subtract)
                u_sb = sb_pool.tile([B_loc, 512], f32, tag="u")
                nc.vector.tensor_mul(u_sb[:, :cw], n_sb[:, :cw], omz_sb[:, cs])
                nc.vector.tensor_add(h_bm[:, cs], u_sb[:, :cw], zh_sb[:, cs])

            if "tp" not in ablate:
                tp = ps_pool.tile([P, KH * B_loc], f32, tag="tp")
                for cc in range(KH):
                    nc.tensor.transpose(tp[:, cc * B_loc:(cc + 1) * B_loc],
                                        h_bm[:, cc * P:(cc + 1) * P], ident[:])
                nc.vector.tensor_copy(hT[:], tp[:])

            # out = h' @ W_out.T  (also next x)
            if "out" in ablate:
                return
            out_ps = ps_pool.tile([B_loc, O], f32, tag="out")
            first = True
            if has_bias:
                bias_row(out_ps[:, :], 4 * H, O, True)
                first = False
            for k in range(KH):
                nc.tensor.matmul(out_ps[:], hT[:, k, :],
                                 WoutT[:, k, :],
                                 start=first and k == 0, stop=k == KH - 1)
            nc.vector.tensor_copy(ybuf[:, j, :], out_ps[:])
            # x.T for next step
            xp = ps_pool.tile([P, KO * B_loc], f32, tag="xp")
            for c in range(KO):
                nc.tensor.transpose(xp[:, c * B_loc:(c + 1) * B_loc],
                                    ybuf[:, j, c * P:(c + 1) * P], ident[:])
            nc.vector.tensor_copy(xT[:], xp[:])

        def block(iv0, unroll):
            assert unroll == UNROLL
            for j in range(UNROLL):
                step(j, iv0)
            nc.sync.dma_start(y_d[:, bass.ds(iv0, UNROLL), :], ybuf[:])

        if static:
            for b in range(T // UNROLL):
                block(b * UNROLL, UNROLL)
        elif repeat > 1:
            # timing mode: amplify the scan; y written at fixed offset 0
            def block0(iv0, unroll):
                for j in range(UNROLL):
                    step(j, 0)
                nc.sync.dma_start(y_d[:, 0:UNROLL, :], ybuf[:])
            tc.For_i_unrolled_general(
                start=0, end=T * repeat, step=1,
                unrollable_body=block0, max_unroll=UNROLL,
                hint_engines=(mybir.EngineType.PE,),
            )
        else:
            tc.For_i_unrolled_general(
                start=0, end=T, step=1,
                unrollable_body=block, max_unroll=UNROLL,
                hint_engines=(mybir.EngineType.PE,),
            )
        acc_pool.release()
        ps_pool.release()
        sb_pool.release()

    nc.compile()
    return nc


_CACHE = {}


def _round_f32r(a):
    """Round-to-nearest-even to fp32r (11 mantissa bits kept)."""
    b = np.ascontiguousarray(a, dtype=np.float32).view(np.uint32)
    lsb = (b >> np.uint32(12)) & np.uint32(1)
    out = (b + np.uint32(0x7FF) + lsb) & np.uint32(0xFFFFF000)
    return out.view(np.float32)


def _prepare(z, W_lh, b_lh, W_ih, b_ih, W_hh, b_hh, W_out, b_out, seq_len):
    z = np.asarray(z, dtype=np.float32)
    W_lh = np.asarray(W_lh, dtype=np.float32)
    W_ih = np.asarray(W_ih, dtype=np.float32)
    W_hh = np.asarray(W_hh, dtype=np.float32)
    W_out = np.asarray(W_out, dtype=np.float32)
    b_lh = np.asarray(b_lh, dtype=np.float32)
    b_ih = np.asarray(b_ih, dtype=np.float32)
    b_hh = np.asarray(b_hh, dtype=np.float32)
    b_out = np.asarray(b_out, dtype=np.float32)
    T = int(seq_len)
    B, L = z.shape
    H = W_hh.shape[1]
    O = W_out.shape[0]
    H3 = 3 * H
    B_loc = B // N_CORES
    has_bias = any(np.any(b) for b in (b_lh, b_ih, b_hh, b_out))

    key = (T, B_loc, L, H, O, has_bias)
    if key not in _CACHE:
        _CACHE[key] = _build(*key)
    nc = _CACHE[key]
    import ml_dtypes
    bf16 = ml_dtypes.bfloat16

    def chunked(WT, k):  # [KD*P, N] -> [P, KD, N]
        KD = WT.shape[0] // P
        return np.ascontiguousarray(
            WT.reshape(KD, P, WT.shape[1]).transpose(1, 0, 2))

    common = {
        "WlhT": chunked(W_lh.T, L // P).astype(bf16),
        "WihT": chunked(W_ih.T, 2).astype(bf16),
        "WhhT": chunked(W_hh.T, H // P).astype(bf16),
        "WoutT": chunked(W_out.T, H // P).astype(bf16),
    }
    if has_bias:
        brz = (b_ih + b_hh)[:2 * H]
        common["bias"] = np.concatenate(
            [brz, b_ih[2 * H:], b_hh[2 * H:], b_out, b_lh]
        ).reshape(1, -1).astype(bf16)

    in_maps = []
    for c in range(N_CORES):
        m = dict(common)
        m["zT"] = np.ascontiguousarray(z[c * B_loc:(c + 1) * B_loc].T).astype(bf16)
        in_maps.append(m)
    return nc, in_maps


def kernel(**inputs):
    from concourse import bass_utils

    nc, in_maps = _prepare(**inputs)
    res = bass_utils.run_bass_kernel_spmd(
        nc, in_maps, core_ids=list(range(N_CORES)))
    y = np.concatenate([r["y"] for r in res.results], axis=0)
    return y


def run_traced(inputs):
    from concourse import bass_utils

    nc, in_maps = _prepare(**inputs)
    return bass_utils.run_bass_kernel_spmd(
        nc, in_maps, core_ids=list(range(N_CORES)), trace=True)
